# revision 26
# baseline (speedup 1.0000x reference)
"""ConvBert LightConv kernel v11 for Trainium2 (Bass/Tile), batch-parallel on 8
cores. Fully fused: no DRAM scratch, no separate prepass.

out[b,s,h,c] = sum_j softmax_j(filters[b,s,h,:])[j] * x_pad[b, s+j-4, h*64+c]

v12 over v4-baseline (356us -> ~155us):
- ALL tap products on DVE in one bf16 2x-mode op per tile (NDV=9). The Pool
  (gpsimd) engine's measured wall cost for these broadcast-AP products was
  ~4x the DVE rate on HW, so any Pool product share LOST time; Pool now only
  does the tiny softmax normalize.
- bf16 products on both operands (vs f32): x cast once per tile on ACT;
  filters evacuated as duplicated pairs (below) so DVE runs in 2x mode.
- PAIR-TILE filter pipeline: tiles (2p, 2p+1) share one exp, one reduce, one
  reciprocal, one normalize and 9 stagger matmuls (N=24 instead of 2x9 at
  N=12), plus 2 instead of 4 pair-evacs -- the per-tile instruction count
  drops ~20%, relieving the dispatch-bound PE/ACT sequencers.
- backhalf identity weights padded to 128 cols (FWL-eligible LDWEIGHTS).
- deep tile-pool buffering (6-8 bufs on small pools) so DMA prefetch and
  the filter pipeline never stall the in-order engine queues.
- bench loop body unrolled 4x inside tc.For_i (the loop back-edge costs
  ~13us of pipeline drain/refill per iteration; unrolling amortizes it).

The staggered softmax filters are evacuated from PSUM as DUPLICATED PAIRS
fn_s4[p, j, s, h, 2] (two ACT copies) so the DVE product's filter operand has
an innermost step-1 pair dim -> hardware auto-selects the 2x_1P perf mode
(16-bit, step 1, 4B aligned) despite the c-broadcast, halving DVE product
time. HW-verified exact vs bf16 numpy (microbench).

Per-core, per pair of tiles (TW=120 output tokens each, 35 tiles):
  x_t  [128,768] f32 <- x rows [t0-4, t0+124) per tile   (halo for the taps)
  x_b  [128,768] bf16 (ACT cast) per tile
  fr2  [128,2,108] f32 <- f rows [t0-8, t0+120) per member (stagger halo)
  softmax shared: exp (ACT, 216 cols), reduce+recip (DVE), normalize -> bf16
    fnb2[p,j,s,h] (Pool, strided read).
  stagger via PE: per tap j one matmul, lhsT = B[:, 16-j : 144-j] (column
    slice of one inline band matrix B[k,c]=[k==c-8], bf16), rhs = both
    members' filters (N=24): fn_ps2[k,j,s,h] = fn[t0_s+k-j,h,j].
  evac fn_s4 [128,9,2,12,2] bf16 <- fn_ps2 twice (ACT; strided pair writes).
  per member tile: products P[k,j,hc] = x_b[k,hc] * fn_s4[k,j,s,h,{0,1}] --
    DVE takes taps [0,NDV) in one op with the pair AP; Pool (gpsimd) takes
    taps [NDV,9) with a plain c-broadcast AP. Separate tensors p_a/p_b.
  shift-sum: out[t0+m] = sum_j P[m+j,j] -- lhsT = ident[:, j:j+128] column
    slices of one 128x136 bf16 identity, 18 accumulating PE matmuls into
    PSUM (exact: weights 0/1), in 512+256 column chunks.
  evac o_t (ACT copy), DMA out.
Zero padding at sequence edges via memset of x_t / fr2 edge rows
(exp(0)=1 keeps softmax finite; stagger zeros + x zeros keep PE NaN-free).
"""

import os
import sys

import numpy as np

for _p in ("/opt/trn_rl_repo",):
    if _p not in sys.path:
        sys.path.insert(0, _p)

B, S, D = 8, 4096, 768
H, HD, KS = 12, 64, 9
PAD = KS // 2  # 4
TW = 120  # output tokens per main tile
NT = (S + TW - 1) // TW  # 35 tiles; last covers 16 tokens

_CACHE = {}


def _build_program(loop_n: int | None = None):
    """loop_n=None: single-shot program (used by kernel()). loop_n=K: the
    whole kernel body runs K times under a tc.For_i hardware loop — one NEFF
    execution = K complete DRAM->DRAM kernel executions (bench)."""
    import concourse.bass as bass
    import concourse.tile as tile
    from concourse import mybir

    f32 = mybir.dt.float32
    bf16 = mybir.dt.bfloat16

    NDV = int(os.environ.get("LC_V7_NDV", "9"))  # taps on DVE; rest on Pool
    NPOOL = KS - NDV

    nc = bass.Bass()
    x_d = nc.dram_tensor("x", [S, D], f32, kind="ExternalInput")
    f_d = nc.dram_tensor("f", [S, H * KS], f32, kind="ExternalInput")
    o_d = nc.dram_tensor("o", [S, D], f32, kind="ExternalOutput")

    # ident[k,m] = [k==m] padded to 136 cols; backhalf lhsT_j = ident[:, j:j+128]
    # (128-wide bf16 weights -> compiler enables FWL: ~2x faster LDWEIGHTS)
    id_np = np.zeros((128, 136), dtype=np.float32)
    id_np[:, :128] = np.eye(128, dtype=np.float32)
    id_d = nc.inline_tensor(id_np, name="ident")
    # band B[k,c] = [k == c-8]; stagger lhsT_j = B[:, 16-j : 144-j]
    b_np = np.zeros((128, 144), dtype=np.float32)
    for k in range(128):
        b_np[k, k + 8] = 1.0
    b_d = nc.inline_tensor(b_np, name="band")

    with tile.TileContext(nc) as tc:
        with (
            tc.tile_pool(name="singles", bufs=1) as singles,
            tc.tile_pool(name="fin", bufs=6) as fin,
            tc.tile_pool(name="xin", bufs=8) as xin,
            tc.tile_pool(name="xbp", bufs=8) as xbp,
            tc.tile_pool(name="sfx", bufs=8) as sfx,
            tc.tile_pool(name="fst", bufs=8) as fst,
            tc.tile_pool(name="prod", bufs=4) as prod,
            tc.tile_pool(name="prodb", bufs=4) as prodb,
            tc.tile_pool(name="outs", bufs=8) as outs,
            tc.tile_pool(name="psf", bufs=2, space="PSUM") as psf,
            tc.tile_pool(name="ps", bufs=3, space="PSUM") as ps,
        ):
            id_f32 = singles.tile([128, 136], f32)
            nc.sync.dma_start(out=id_f32, in_=id_d[:, :])
            id_sb = singles.tile([128, 136], bf16)
            nc.vector.tensor_copy(id_sb, id_f32)
            b_f32 = singles.tile([128, 144], f32)
            nc.sync.dma_start(out=b_f32, in_=b_d[:, :])
            b_sb = singles.tile([128, 144], bf16)
            nc.vector.tensor_copy(b_sb, b_f32)

            Copy = mybir.ActivationFunctionType.Copy
            Exp = mybir.ActivationFunctionType.Exp

            def _emit_prep_pair(p):
                """Shared filter pipeline + per-member x prefetch for tiles
                (2p, 2p+1). Returns per-member (x_b, fn_s4, s, tw, t0)."""
                members = [t for t in (2 * p, 2 * p + 1) if t < NT]

                xbs = []
                for t in members:
                    t0 = TW * t
                    u0 = t0 - PAD
                    x_t = xin.tile([128, D], f32, tag="x_t")
                    if t == 0:
                        nc.vector.memset(x_t[0:PAD, :], 0.0)
                        nc.sync.dma_start(
                            out=x_t[PAD:128, :], in_=x_d[0 : 128 - PAD, :]
                        )
                    elif u0 + 128 > S:
                        nv = S - u0
                        nc.vector.memset(x_t, 0.0)
                        nc.sync.dma_start(out=x_t[0:nv, :], in_=x_d[u0:S, :])
                    else:
                        nc.sync.dma_start(out=x_t, in_=x_d[u0 : u0 + 128, :])
                    x_b = xbp.tile([128, D], bf16, tag="x_b")
                    nc.scalar.activation(x_b, x_t, Copy)
                    xbs.append(x_b)

                fr2 = fin.tile([128, 2, H * KS], f32, tag="fr2")
                for s, t in enumerate(members):
                    t0 = TW * t
                    f0 = t0 - 2 * PAD  # first f row (stagger halo)
                    if t == 0:
                        nc.gpsimd.memset(fr2[0 : 2 * PAD, 0, :], 0.0)
                        nc.scalar.dma_start(
                            out=fr2[2 * PAD : 128, 0, :],
                            in_=f_d[0 : 128 - 2 * PAD, :],
                        )
                    elif f0 + 128 > S:
                        nv = S - f0
                        nc.gpsimd.memset(fr2[:, s, :], 0.0)
                        nc.scalar.dma_start(out=fr2[0:nv, s, :], in_=f_d[f0:S, :])
                    else:
                        nc.scalar.dma_start(
                            out=fr2[:, s, :], in_=f_d[f0 : f0 + 128, :]
                        )
                if len(members) == 1:
                    nc.gpsimd.memset(fr2[:, 1, :], 0.0)

                e2 = sfx.tile([128, 2, H * KS], f32, tag="e2")
                nc.scalar.activation(e2, fr2, Exp)
                z2 = sfx.tile([128, 2, H], f32, tag="z2")
                nc.vector.tensor_reduce(
                    out=z2,
                    in_=e2.rearrange("p s (h j) -> p s h j", j=KS),
                    axis=mybir.AxisListType.X,
                    op=mybir.AluOpType.add,
                )
                r2 = sfx.tile([128, 2, H], f32, tag="r2")
                nc.vector.reciprocal(r2, z2)
                fnb2 = sfx.tile([128, KS, 2, H], bf16, tag="fnb2")
                nc.gpsimd.tensor_mul(
                    fnb2,
                    e2.rearrange("p s (h j) -> p j s h", j=KS),
                    r2.unsqueeze(1).broadcast_to([128, KS, 2, H]),
                )

                fn_ps2 = psf.tile([128, KS, 2, H], f32, tag="fn_ps2")
                for j in range(KS):
                    nc.tensor.matmul(
                        fn_ps2[:, j, :, :],
                        b_sb[:, 16 - j : 144 - j],
                        fnb2[:, j, :, :],
                        start=True,
                        stop=True,
                    )

                # pairs: fn_s4[p,s,j,h,0] == fn_s4[p,s,j,h,1]; s OUTERMOST so
                # the per-member slice has v10's mergeable (j,h) layout (3 free
                # dims after AP opt -- walrus TENSOR3D limit).
                fn_s4 = fst.tile([128, 2, KS, H, 2], bf16, tag="fn_s4")
                nc.scalar.activation(
                    fn_s4[:, :, :, :, 0].rearrange("p s j h -> p j s h"),
                    fn_ps2, Copy)
                nc.scalar.activation(
                    fn_s4[:, :, :, :, 1].rearrange("p s j h -> p j s h"),
                    fn_ps2, Copy)

                return [
                    (xbs[s], fn_s4, s, min(TW, S - TW * t), TW * t)
                    for s, t in enumerate(members)
                ]

            def _emit_main(prep):
                x_b, fn_s4, s, tw, t0 = prep
                # products -> p_a (DVE taps [0,NDV)), p_b (Pool taps)
                p_a = prod.tile([128, NDV, D], bf16, tag="p_a")
                C2 = HD // 2
                x_v = x_b.rearrange("p (h c2 two) -> p h c2 two", h=H, c2=C2)
                NGRP = int(os.environ.get("LC_V13_GRP", "2"))
                bounds = [NDV * g // NGRP for g in range(NGRP + 1)]
                for j0, j1 in zip(bounds[:-1], bounds[1:]):
                    nj = j1 - j0
                    if nj == 0:
                        continue
                    nc.vector.tensor_mul(
                        p_a[:, j0:j1].rearrange(
                            "p j (h c2 two) -> p j h c2 two", h=H, c2=C2
                        ),
                        x_v.unsqueeze(1).broadcast_to([128, nj, H, C2, 2]),
                        fn_s4[:, s : s + 1, j0:j1]
                        .rearrange("p s j h two -> p (s j) h two")
                        .unsqueeze(3)
                        .broadcast_to([128, nj, H, C2, 2]),
                    )
                if NPOOL:
                    p_b = prodb.tile([128, NPOOL, D], bf16, tag="p_b")
                    x_hc = x_b.rearrange("p (h c) -> p h c", c=HD)
                    SPL = int(os.environ.get("LC_V11_SPLIT", "512"))  # cols
                    # of tap NDV done by DVE (head-aligned); Pool does the
                    # remaining heads of tap NDV plus taps NDV+1..KS-1.
                    HSPL = SPL // HD
                    if HSPL:
                        nc.vector.tensor_mul(
                            p_b[:, 0, 0:SPL].rearrange(
                                "p (h c2 two) -> p h c2 two", h=HSPL, c2=C2
                            ),
                            x_v[:, 0:HSPL],
                            fn_s4[:, s : s + 1, NDV : NDV + 1, 0:HSPL]
                            .rearrange("p s j h two -> p (s j h) two")
                            .unsqueeze(2)
                            .broadcast_to([128, HSPL, C2, 2]),
                        )
                    if HSPL < H:
                        nc.gpsimd.tensor_mul(
                            p_b[:, 0, SPL:].rearrange(
                                "p (h c) -> p h c", c=HD
                            ),
                            x_hc[:, HSPL:, :],
                            fn_s4[:, s : s + 1, NDV : NDV + 1, HSPL:, 0]
                            .rearrange("p s j h -> p (s j h)")
                            .unsqueeze(2)
                            .broadcast_to([128, H - HSPL, HD]),
                        )
                    if NPOOL > 1:
                        nc.gpsimd.tensor_mul(
                            p_b[:, 1:, :].rearrange(
                                "p j (h c) -> p j h c", c=HD
                            ),
                            x_hc.unsqueeze(1).broadcast_to(
                                [128, NPOOL - 1, H, HD]
                            ),
                            fn_s4[:, s : s + 1, NDV + 1 : KS, :, 0]
                            .rearrange("p s j h -> p (s j) h")
                            .unsqueeze(3)
                            .broadcast_to([128, NPOOL - 1, H, HD]),
                        )
                o_ps = ps.tile([128, D], f32, tag="o_ps")
                for j in range(KS):
                    lhsT = id_sb[:, j : j + 128]
                    rhs_t = p_a if j < NDV else p_b
                    jj = j if j < NDV else j - NDV
                    for n0, n1 in ((0, 512), (512, D)):
                        nc.tensor.matmul(
                            o_ps[0:128, n0:n1],
                            lhsT,
                            rhs_t[:, jj, n0:n1],
                            start=(j == 0),
                            stop=(j == KS - 1),
                        )

                o_t = outs.tile([128, D], f32, tag="o_t")
                nc.scalar.activation(o_t[0:tw, :], o_ps[0:tw, :], Copy)
                nc.sync.dma_start(out=o_d[t0 : t0 + tw, :], in_=o_t[0:tw, :])

            def _kernel_body():
                # prep(p+1) is emitted BETWEEN the two mains of pair p so its
                # 9 stagger matmuls queue ahead of pair p's second backhalf
                # block on the in-order PE queue -- the filter chain for the
                # next pair no longer gates the DVE products late in the
                # period.
                npairs = (NT + 1) // 2
                preps = _emit_prep_pair(0)
                for p in range(npairs):
                    _emit_main(preps[0])
                    nxt = _emit_prep_pair(p + 1) if p + 1 < npairs else None
                    for m in preps[1:]:
                        _emit_main(m)
                    preps = nxt

            UNROLL = int(os.environ.get("LC_V9_UNROLL", "16"))
            if loop_n is None:
                _kernel_body()
            else:
                with tc.For_i(0, loop_n):
                    for _ in range(UNROLL):
                        _kernel_body()

    _split_hwdge_multi_waits(nc)
    return nc


def _split_hwdge_multi_waits(nc):
    """walrus's HWDGE DMA trigger (PSEUDO_DMA_DIRECT2D) rejects >1 sync wait
    on a DMACopy. Move all but one wait onto a NoOp inserted right before the
    DMA on the same (sequencer) engine — identical semantics, since the
    sequencer executes both in order before triggering the descriptor."""
    from concourse import mybir

    nsplit = 0
    for fn in nc.m.functions:
        for blk in fn.blocks:
            out = []
            for ins in blk.instructions:
                si = ins.sync_info
                if si is not None and len(si.on_wait) > 1:
                    for wi, w in enumerate(si.on_wait[:-1]):
                        nop = mybir.InstNoOp(
                            name=f"{ins.name}_waitsplit{wi}",
                            engine=ins.engine,
                            sync_info=mybir.SyncInfo(on_wait=[w], on_update=[]),
                        )
                        out.append(nop)
                    ins.sync_info = mybir.SyncInfo(
                        on_wait=list(si.on_wait[-1:]),
                        on_update=list(si.on_update),
                    )
                    nsplit += 1
                out.append(ins)
            blk.instructions = out
    if nsplit and os.environ.get("LC_DEBUG"):
        print(f"_split_hwdge_multi_waits: split {nsplit} DMAs")


def kernel(inputs: np.ndarray, filters: np.ndarray) -> np.ndarray:
    from concourse.bass_utils import run_bass_kernel_spmd

    if "nc" not in _CACHE:
        _CACHE["nc"] = _build_program()
    nc = _CACHE["nc"]

    inputs = np.ascontiguousarray(np.asarray(inputs, dtype=np.float32))
    filters = np.ascontiguousarray(np.asarray(filters, dtype=np.float32))

    in_maps = [{"x": inputs[c], "f": filters[c]} for c in range(B)]

    res = run_bass_kernel_spmd(nc, in_maps, core_ids=list(range(B)), trace=False)

    out = np.stack([res.results[c]["o"] for c in range(B)], axis=0)
    return out.reshape(B, S, H, HD)

def bench(
    inputs: np.ndarray, filters: np.ndarray, reps: int = 20, loop_n: int = 1000
) -> float:
    """Steady-state device benchmark. One NEFF launch executes the complete
    kernel (prepass + main pass, full DRAM->DRAM dataflow) ``loop_n`` times
    under a tc.For_i hardware loop; ``reps`` launches are timed back-to-back
    after a warm-up launch. Returns mean seconds per kernel execution —
    launch/transfer overheads are amortized over reps*loop_n executions."""
    import time

    import jax
    from jax.experimental.shard_map import shard_map
    from jax.sharding import Mesh, PartitionSpec

    import concourse.mybir as mybir
    from concourse import bass2jax

    unroll = int(os.environ.get("LC_V9_UNROLL", "16"))
    loop_n = max(1, loop_n // unroll)
    key = f"nc_loop{loop_n}_u{unroll}"
    if key not in _CACHE:
        _CACHE[key] = _build_program(loop_n=loop_n)
    nc = _CACHE[key]
    loop_n = loop_n * unroll  # per-kernel normalization below
    bass2jax.install_neuronx_cc_hook()

    part_name = nc.partition_id_tensor.name if nc.partition_id_tensor else None
    in_names, out_names, out_avals, zero_outs = [], [], [], []
    for alloc in nc.m.functions[0].allocations:
        if not isinstance(alloc, mybir.MemoryLocationSet):
            continue
        name = alloc.memorylocations[0].name
        if alloc.kind == "ExternalInput":
            if name != part_name:
                in_names.append(name)
        elif alloc.kind == "ExternalOutput":
            out_names.append(name)
            shape = tuple(alloc.tensor_shape)
            dtype = mybir.dt.np(alloc.dtype)
            out_avals.append(jax.core.ShapedArray(shape, dtype))
            zero_outs.append(np.zeros(shape, dtype))
    n_params = len(in_names)
    all_names = in_names + out_names
    if part_name is not None:
        all_names = all_names + [part_name]

    def _body(*args):
        operands = list(args)
        if part_name is not None:
            operands.append(bass2jax.partition_id_tensor())
        outs = bass2jax._bass_exec_p.bind(
            *operands,
            out_avals=tuple(out_avals),
            in_names=tuple(all_names),
            out_names=tuple(out_names),
            lowering_input_output_aliases=(),
            sim_require_finite=True,
            sim_require_nnan=True,
            nc=nc,
        )
        return tuple(outs)

    devices = jax.devices()[:B]
    mesh = Mesh(np.asarray(devices), ("core",))
    nin = n_params + len(out_names)
    fn = jax.jit(
        shard_map(
            _body,
            mesh=mesh,
            in_specs=(PartitionSpec("core"),) * nin,
            out_specs=(PartitionSpec("core"),) * len(out_names),
            check_rep=False,
        ),
        keep_unused=True,
    )
    per_core = {"x": inputs.astype(np.float32), "f": filters.astype(np.float32)}
    concat_in = [
        np.concatenate([per_core[n][c] for c in range(B)], axis=0) for n in in_names
    ]
    concat_zero = [
        np.zeros((B * z.shape[0], *z.shape[1:]), z.dtype) for z in zero_outs
    ]
    sharding = jax.sharding.NamedSharding(mesh, PartitionSpec("core"))
    dev_args = [jax.device_put(a, sharding) for a in concat_in + concat_zero]

    out = fn(*dev_args)  # compile + warm
    jax.block_until_ready(out)
    t0 = time.perf_counter()
    for _ in range(reps):
        out = fn(*dev_args)
    jax.block_until_ready(out)
    t1 = time.perf_counter()
    return (t1 - t0) / (reps * loop_n)


if __name__ == "__main__":
    rng = np.random.default_rng(0)
    x = rng.standard_normal((B, S, D), dtype=np.float32)
    f = rng.standard_normal((B, S, H * KS), dtype=np.float32)
    o = kernel(x, f)
    print(o.shape, o.dtype)


# revision 27
# speedup vs baseline: 1.1157x; 1.1157x over previous
"""ConvBert LightConv kernel v11 for Trainium2 (Bass/Tile), batch-parallel on 8
cores. Fully fused: no DRAM scratch, no separate prepass.

out[b,s,h,c] = sum_j softmax_j(filters[b,s,h,:])[j] * x_pad[b, s+j-4, h*64+c]

v12 over v4-baseline (356us -> ~155us):
- ALL tap products on DVE in one bf16 2x-mode op per tile (NDV=9). The Pool
  (gpsimd) engine's measured wall cost for these broadcast-AP products was
  ~4x the DVE rate on HW, so any Pool product share LOST time; Pool now only
  does the tiny softmax normalize.
- bf16 products on both operands (vs f32): x cast once per tile on ACT;
  filters evacuated as duplicated pairs (below) so DVE runs in 2x mode.
- PAIR-TILE filter pipeline: tiles (2p, 2p+1) share one exp, one reduce, one
  reciprocal, one normalize and 9 stagger matmuls (N=24 instead of 2x9 at
  N=12), plus 2 instead of 4 pair-evacs -- the per-tile instruction count
  drops ~20%, relieving the dispatch-bound PE/ACT sequencers.
- backhalf identity weights padded to 128 cols (FWL-eligible LDWEIGHTS).
- deep tile-pool buffering (6-8 bufs on small pools) so DMA prefetch and
  the filter pipeline never stall the in-order engine queues.
- bench loop body unrolled 4x inside tc.For_i (the loop back-edge costs
  ~13us of pipeline drain/refill per iteration; unrolling amortizes it).

The staggered softmax filters are evacuated from PSUM as DUPLICATED PAIRS
fn_s4[p, j, s, h, 2] (two ACT copies) so the DVE product's filter operand has
an innermost step-1 pair dim -> hardware auto-selects the 2x_1P perf mode
(16-bit, step 1, 4B aligned) despite the c-broadcast, halving DVE product
time. HW-verified exact vs bf16 numpy (microbench).

Per-core, per pair of tiles (TW=120 output tokens each, 35 tiles):
  x_t  [128,768] f32 <- x rows [t0-4, t0+124) per tile   (halo for the taps)
  x_b  [128,768] bf16 (ACT cast) per tile
  fr2  [128,2,108] f32 <- f rows [t0-8, t0+120) per member (stagger halo)
  softmax shared: exp (ACT, 216 cols), reduce+recip (DVE), normalize -> bf16
    fnb2[p,j,s,h] (Pool, strided read).
  stagger via PE: per tap j one matmul, lhsT = B[:, 16-j : 144-j] (column
    slice of one inline band matrix B[k,c]=[k==c-8], bf16), rhs = both
    members' filters (N=24): fn_ps2[k,j,s,h] = fn[t0_s+k-j,h,j].
  evac fn_s4 [128,9,2,12,2] bf16 <- fn_ps2 twice (ACT; strided pair writes).
  per member tile: products P[k,j,hc] = x_b[k,hc] * fn_s4[k,j,s,h,{0,1}] --
    DVE takes taps [0,NDV) in one op with the pair AP; Pool (gpsimd) takes
    taps [NDV,9) with a plain c-broadcast AP. Separate tensors p_a/p_b.
  shift-sum: out[t0+m] = sum_j P[m+j,j] -- lhsT = ident[:, j:j+128] column
    slices of one 128x136 bf16 identity, 18 accumulating PE matmuls into
    PSUM (exact: weights 0/1), in 512+256 column chunks.
  evac o_t (ACT copy), DMA out.
Zero padding at sequence edges via memset of x_t / fr2 edge rows
(exp(0)=1 keeps softmax finite; stagger zeros + x zeros keep PE NaN-free).
"""

import os
import sys

import numpy as np

for _p in ("/opt/trn_rl_repo",):
    if _p not in sys.path:
        sys.path.insert(0, _p)

B, S, D = 8, 4096, 768
H, HD, KS = 12, 64, 9
PAD = KS // 2  # 4
TW = 120  # output tokens per main tile
NT = (S + TW - 1) // TW  # 35 tiles; last covers 16 tokens

_CACHE = {}


def _build_program(loop_n: int | None = None):
    """loop_n=None: single-shot program (used by kernel()). loop_n=K: the
    whole kernel body runs K times under a tc.For_i hardware loop — one NEFF
    execution = K complete DRAM->DRAM kernel executions (bench)."""
    import concourse.bass as bass
    import concourse.tile as tile
    from concourse import mybir

    f32 = mybir.dt.float32
    bf16 = mybir.dt.bfloat16

    NDV = int(os.environ.get("LC_V7_NDV", "9"))  # taps on DVE; rest on Pool
    NPOOL = KS - NDV

    nc = bass.Bass()
    x_d = nc.dram_tensor("x", [S, D], f32, kind="ExternalInput")
    f_d = nc.dram_tensor("f", [S, H * KS], f32, kind="ExternalInput")
    o_d = nc.dram_tensor("o", [S, D], f32, kind="ExternalOutput")

    # ident[k,m] = [k==m] padded to 136 cols; backhalf lhsT_j = ident[:, j:j+128]
    # (128-wide bf16 weights -> compiler enables FWL: ~2x faster LDWEIGHTS)
    id_np = np.zeros((128, 136), dtype=np.float32)
    id_np[:, :128] = np.eye(128, dtype=np.float32)
    id_d = nc.inline_tensor(id_np, name="ident")
    # band B[k,c] = [k == c-8]; stagger lhsT_j = B[:, 16-j : 144-j]
    b_np = np.zeros((128, 144), dtype=np.float32)
    for k in range(128):
        b_np[k, k + 8] = 1.0
    b_d = nc.inline_tensor(b_np, name="band")

    with tile.TileContext(nc) as tc:
        with (
            tc.tile_pool(name="singles", bufs=1) as singles,
            tc.tile_pool(name="fin", bufs=6) as fin,
            tc.tile_pool(name="xin", bufs=8) as xin,
            tc.tile_pool(name="xbp", bufs=8) as xbp,
            tc.tile_pool(name="sfx", bufs=8) as sfx,
            tc.tile_pool(name="fst", bufs=8) as fst,
            tc.tile_pool(name="prod", bufs=4) as prod,
            tc.tile_pool(name="prodb", bufs=4) as prodb,
            tc.tile_pool(name="outs", bufs=8) as outs,
            tc.tile_pool(name="psf", bufs=2, space="PSUM") as psf,
            tc.tile_pool(name="ps", bufs=3, space="PSUM") as ps,
        ):
            id_f32 = singles.tile([128, 136], f32)
            nc.sync.dma_start(out=id_f32, in_=id_d[:, :])
            id_sb = singles.tile([128, 136], bf16)
            nc.vector.tensor_copy(id_sb, id_f32)
            b_f32 = singles.tile([128, 144], f32)
            nc.sync.dma_start(out=b_f32, in_=b_d[:, :])
            b_sb = singles.tile([128, 144], bf16)
            nc.vector.tensor_copy(b_sb, b_f32)

            Copy = mybir.ActivationFunctionType.Copy
            Exp = mybir.ActivationFunctionType.Exp

            def _emit_prep_pair(p):
                """Shared filter pipeline + per-member x prefetch for tiles
                (2p, 2p+1). Returns per-member (x_b, fn_s4, s, tw, t0)."""
                members = [t for t in (2 * p, 2 * p + 1) if t < NT]

                xbs = []
                for t in members:
                    t0 = TW * t
                    u0 = t0 - PAD
                    x_t = xin.tile([128, D], f32, tag="x_t")
                    if t == 0:
                        nc.vector.memset(x_t[0:PAD, :], 0.0)
                        nc.sync.dma_start(
                            out=x_t[PAD:128, :], in_=x_d[0 : 128 - PAD, :]
                        )
                    elif u0 + 128 > S:
                        nv = S - u0
                        nc.vector.memset(x_t, 0.0)
                        nc.sync.dma_start(out=x_t[0:nv, :], in_=x_d[u0:S, :])
                    else:
                        nc.sync.dma_start(out=x_t, in_=x_d[u0 : u0 + 128, :])
                    x_b = xbp.tile([128, D], bf16, tag="x_b")
                    nc.scalar.activation(x_b, x_t, Copy)
                    xbs.append(x_b)

                fr2 = fin.tile([128, 2, H * KS], f32, tag="fr2")
                for s, t in enumerate(members):
                    t0 = TW * t
                    f0 = t0 - 2 * PAD  # first f row (stagger halo)
                    if t == 0:
                        nc.gpsimd.memset(fr2[0 : 2 * PAD, 0, :], 0.0)
                        nc.scalar.dma_start(
                            out=fr2[2 * PAD : 128, 0, :],
                            in_=f_d[0 : 128 - 2 * PAD, :],
                        )
                    elif f0 + 128 > S:
                        nv = S - f0
                        nc.gpsimd.memset(fr2[:, s, :], 0.0)
                        nc.scalar.dma_start(out=fr2[0:nv, s, :], in_=f_d[f0:S, :])
                    else:
                        nc.scalar.dma_start(
                            out=fr2[:, s, :], in_=f_d[f0 : f0 + 128, :]
                        )
                if len(members) == 1:
                    nc.gpsimd.memset(fr2[:, 1, :], 0.0)

                e2 = sfx.tile([128, 2, H * KS], f32, tag="e2")
                nc.scalar.activation(e2, fr2, Exp)
                z2 = sfx.tile([128, 2, H], f32, tag="z2")
                nc.vector.tensor_reduce(
                    out=z2,
                    in_=e2.rearrange("p s (h j) -> p s h j", j=KS),
                    axis=mybir.AxisListType.X,
                    op=mybir.AluOpType.add,
                )
                r2 = sfx.tile([128, 2, H], f32, tag="r2")
                nc.vector.reciprocal(r2, z2)
                fnb2 = sfx.tile([128, KS, 2, H], bf16, tag="fnb2")
                nc.gpsimd.tensor_mul(
                    fnb2,
                    e2.rearrange("p s (h j) -> p j s h", j=KS),
                    r2.unsqueeze(1).broadcast_to([128, KS, 2, H]),
                )

                fn_ps2 = psf.tile([128, KS, 2, H], f32, tag="fn_ps2")
                for j in range(KS):
                    nc.tensor.matmul(
                        fn_ps2[:, j, :, :],
                        b_sb[:, 16 - j : 144 - j],
                        fnb2[:, j, :, :],
                        start=True,
                        stop=True,
                    )

                # pairs: fn_s4[p,s,j,h,0] == fn_s4[p,s,j,h,1]; s OUTERMOST so
                # the per-member slice has v10's mergeable (j,h) layout (3 free
                # dims after AP opt -- walrus TENSOR3D limit).
                fn_s4 = fst.tile([128, 2, KS, H, 2], bf16, tag="fn_s4")
                nc.scalar.activation(
                    fn_s4[:, :, :, :, 0].rearrange("p s j h -> p j s h"),
                    fn_ps2, Copy)
                nc.scalar.activation(
                    fn_s4[:, :, :, :, 1].rearrange("p s j h -> p j s h"),
                    fn_ps2, Copy)

                return [
                    (xbs[s], fn_s4, s, min(TW, S - TW * t), TW * t)
                    for s, t in enumerate(members)
                ]

            def _emit_main(prep):
                x_b, fn_s4, s, tw, t0 = prep
                # products -> p_a (DVE taps [0,NDV)), p_b (Pool taps)
                p_a = prod.tile([128, NDV, D], bf16, tag="p_a")
                C2 = HD // 2
                x_v = x_b.rearrange("p (h c2 two) -> p h c2 two", h=H, c2=C2)
                NGRP = int(os.environ.get("LC_V13_GRP", "2"))
                bounds = [NDV * g // NGRP for g in range(NGRP + 1)]
                for j0, j1 in zip(bounds[:-1], bounds[1:]):
                    nj = j1 - j0
                    if nj == 0:
                        continue
                    nc.vector.tensor_mul(
                        p_a[:, j0:j1].rearrange(
                            "p j (h c2 two) -> p j h c2 two", h=H, c2=C2
                        ),
                        x_v.unsqueeze(1).broadcast_to([128, nj, H, C2, 2]),
                        fn_s4[:, s : s + 1, j0:j1]
                        .rearrange("p s j h two -> p (s j) h two")
                        .unsqueeze(3)
                        .broadcast_to([128, nj, H, C2, 2]),
                    )
                if NPOOL:
                    p_b = prodb.tile([128, NPOOL, D], bf16, tag="p_b")
                    x_hc = x_b.rearrange("p (h c) -> p h c", c=HD)
                    SPL = int(os.environ.get("LC_V11_SPLIT", "512"))  # cols
                    # of tap NDV done by DVE (head-aligned); Pool does the
                    # remaining heads of tap NDV plus taps NDV+1..KS-1.
                    HSPL = SPL // HD
                    if HSPL:
                        nc.vector.tensor_mul(
                            p_b[:, 0, 0:SPL].rearrange(
                                "p (h c2 two) -> p h c2 two", h=HSPL, c2=C2
                            ),
                            x_v[:, 0:HSPL],
                            fn_s4[:, s : s + 1, NDV : NDV + 1, 0:HSPL]
                            .rearrange("p s j h two -> p (s j h) two")
                            .unsqueeze(2)
                            .broadcast_to([128, HSPL, C2, 2]),
                        )
                    if HSPL < H:
                        nc.gpsimd.tensor_mul(
                            p_b[:, 0, SPL:].rearrange(
                                "p (h c) -> p h c", c=HD
                            ),
                            x_hc[:, HSPL:, :],
                            fn_s4[:, s : s + 1, NDV : NDV + 1, HSPL:, 0]
                            .rearrange("p s j h -> p (s j h)")
                            .unsqueeze(2)
                            .broadcast_to([128, H - HSPL, HD]),
                        )
                    if NPOOL > 1:
                        nc.gpsimd.tensor_mul(
                            p_b[:, 1:, :].rearrange(
                                "p j (h c) -> p j h c", c=HD
                            ),
                            x_hc.unsqueeze(1).broadcast_to(
                                [128, NPOOL - 1, H, HD]
                            ),
                            fn_s4[:, s : s + 1, NDV + 1 : KS, :, 0]
                            .rearrange("p s j h -> p (s j) h")
                            .unsqueeze(3)
                            .broadcast_to([128, NPOOL - 1, H, HD]),
                        )
                o_ps = ps.tile([128, D], f32, tag="o_ps")
                for j in range(KS):
                    lhsT = id_sb[:, j : j + 128]
                    rhs_t = p_a if j < NDV else p_b
                    jj = j if j < NDV else j - NDV
                    for n0, n1 in ((0, 512), (512, D)):
                        nc.tensor.matmul(
                            o_ps[0:128, n0:n1],
                            lhsT,
                            rhs_t[:, jj, n0:n1],
                            start=(j == 0),
                            stop=(j == KS - 1),
                        )

                o_t = outs.tile([128, D], f32, tag="o_t")
                nc.scalar.activation(o_t[0:tw, :], o_ps[0:tw, :], Copy)
                nc.sync.dma_start(out=o_d[t0 : t0 + tw, :], in_=o_t[0:tw, :])

            def _kernel_body():
                for p in range((NT + 1) // 2):
                    for prep in _emit_prep_pair(p):
                        _emit_main(prep)

            UNROLL = int(os.environ.get("LC_V9_UNROLL", "16"))
            if loop_n is None:
                _kernel_body()
            else:
                with tc.For_i(0, loop_n):
                    for _ in range(UNROLL):
                        _kernel_body()

    _split_hwdge_multi_waits(nc)
    return nc


def _split_hwdge_multi_waits(nc):
    """walrus's HWDGE DMA trigger (PSEUDO_DMA_DIRECT2D) rejects >1 sync wait
    on a DMACopy. Move all but one wait onto a NoOp inserted right before the
    DMA on the same (sequencer) engine — identical semantics, since the
    sequencer executes both in order before triggering the descriptor."""
    from concourse import mybir

    nsplit = 0
    for fn in nc.m.functions:
        for blk in fn.blocks:
            out = []
            for ins in blk.instructions:
                si = ins.sync_info
                if si is not None and len(si.on_wait) > 1:
                    for wi, w in enumerate(si.on_wait[:-1]):
                        nop = mybir.InstNoOp(
                            name=f"{ins.name}_waitsplit{wi}",
                            engine=ins.engine,
                            sync_info=mybir.SyncInfo(on_wait=[w], on_update=[]),
                        )
                        out.append(nop)
                    ins.sync_info = mybir.SyncInfo(
                        on_wait=list(si.on_wait[-1:]),
                        on_update=list(si.on_update),
                    )
                    nsplit += 1
                out.append(ins)
            blk.instructions = out
    if nsplit and os.environ.get("LC_DEBUG"):
        print(f"_split_hwdge_multi_waits: split {nsplit} DMAs")


def kernel(inputs: np.ndarray, filters: np.ndarray) -> np.ndarray:
    from concourse.bass_utils import run_bass_kernel_spmd

    if "nc" not in _CACHE:
        _CACHE["nc"] = _build_program()
    nc = _CACHE["nc"]

    inputs = np.ascontiguousarray(np.asarray(inputs, dtype=np.float32))
    filters = np.ascontiguousarray(np.asarray(filters, dtype=np.float32))

    in_maps = [{"x": inputs[c], "f": filters[c]} for c in range(B)]

    res = run_bass_kernel_spmd(nc, in_maps, core_ids=list(range(B)), trace=False)

    out = np.stack([res.results[c]["o"] for c in range(B)], axis=0)
    return out.reshape(B, S, H, HD)

def bench(
    inputs: np.ndarray, filters: np.ndarray, reps: int = 20, loop_n: int = 1000
) -> float:
    """Steady-state device benchmark. One NEFF launch executes the complete
    kernel (prepass + main pass, full DRAM->DRAM dataflow) ``loop_n`` times
    under a tc.For_i hardware loop; ``reps`` launches are timed back-to-back
    after a warm-up launch. Returns mean seconds per kernel execution —
    launch/transfer overheads are amortized over reps*loop_n executions."""
    import time

    import jax
    from jax.experimental.shard_map import shard_map
    from jax.sharding import Mesh, PartitionSpec

    import concourse.mybir as mybir
    from concourse import bass2jax

    unroll = int(os.environ.get("LC_V9_UNROLL", "16"))
    loop_n = max(1, loop_n // unroll)
    key = f"nc_loop{loop_n}_u{unroll}"
    if key not in _CACHE:
        _CACHE[key] = _build_program(loop_n=loop_n)
    nc = _CACHE[key]
    loop_n = loop_n * unroll  # per-kernel normalization below
    bass2jax.install_neuronx_cc_hook()

    part_name = nc.partition_id_tensor.name if nc.partition_id_tensor else None
    in_names, out_names, out_avals, zero_outs = [], [], [], []
    for alloc in nc.m.functions[0].allocations:
        if not isinstance(alloc, mybir.MemoryLocationSet):
            continue
        name = alloc.memorylocations[0].name
        if alloc.kind == "ExternalInput":
            if name != part_name:
                in_names.append(name)
        elif alloc.kind == "ExternalOutput":
            out_names.append(name)
            shape = tuple(alloc.tensor_shape)
            dtype = mybir.dt.np(alloc.dtype)
            out_avals.append(jax.core.ShapedArray(shape, dtype))
            zero_outs.append(np.zeros(shape, dtype))
    n_params = len(in_names)
    all_names = in_names + out_names
    if part_name is not None:
        all_names = all_names + [part_name]

    def _body(*args):
        operands = list(args)
        if part_name is not None:
            operands.append(bass2jax.partition_id_tensor())
        outs = bass2jax._bass_exec_p.bind(
            *operands,
            out_avals=tuple(out_avals),
            in_names=tuple(all_names),
            out_names=tuple(out_names),
            lowering_input_output_aliases=(),
            sim_require_finite=True,
            sim_require_nnan=True,
            nc=nc,
        )
        return tuple(outs)

    devices = jax.devices()[:B]
    mesh = Mesh(np.asarray(devices), ("core",))
    nin = n_params + len(out_names)
    fn = jax.jit(
        shard_map(
            _body,
            mesh=mesh,
            in_specs=(PartitionSpec("core"),) * nin,
            out_specs=(PartitionSpec("core"),) * len(out_names),
            check_rep=False,
        ),
        keep_unused=True,
    )
    per_core = {"x": inputs.astype(np.float32), "f": filters.astype(np.float32)}
    concat_in = [
        np.concatenate([per_core[n][c] for c in range(B)], axis=0) for n in in_names
    ]
    concat_zero = [
        np.zeros((B * z.shape[0], *z.shape[1:]), z.dtype) for z in zero_outs
    ]
    sharding = jax.sharding.NamedSharding(mesh, PartitionSpec("core"))
    dev_args = [jax.device_put(a, sharding) for a in concat_in + concat_zero]

    out = fn(*dev_args)  # compile + warm
    jax.block_until_ready(out)
    t0 = time.perf_counter()
    for _ in range(reps):
        out = fn(*dev_args)
    jax.block_until_ready(out)
    t1 = time.perf_counter()
    return (t1 - t0) / (reps * loop_n)


if __name__ == "__main__":
    rng = np.random.default_rng(0)
    x = rng.standard_normal((B, S, D), dtype=np.float32)
    f = rng.standard_normal((B, S, H * KS), dtype=np.float32)
    o = kernel(x, f)
    print(o.shape, o.dtype)
